# revision 7
# baseline (speedup 1.0000x reference)
"""Trainium2 Bass kernel for EnvelopeLinearCQN (MLP trunk + envelope top-1 Q selection).

Sharding: tensor-parallel column-split of the two big H1->H2 GEMMs across 8
cores (960 H2-cols each, zero-padded to 1024). Each core computes the full
h1 = relu(x@W1+b1) (cheap, replicated), its column slice of
g = sigmoid(500*(h1@Wg+bg)) and h2 = relu(h1@W2+b2), then the partial
q = (h2*g)_slice @ W3_slice. Partial q [1024,32] is AllReduced on-chip;
each core then does the H() top-1 selection for its own 128 batch rows.

Precision: the x500 sigmoid gate amplifies matmul rounding ~12x in RMS and
the H() argmax flips candidates on small q errors, so plain bf16/fp16 GEMMs
fail. All GEMMs therefore run as 3-pass fp16 hi/lo split-precision
(a@b ~= ah@bh + al@bh + ah@bl with lo terms pre-scaled by 2^11 to dodge fp16
subnormals), giving ~1e-6 relative error (~fp32) at 3x bf16-GEMM cost --
still ~1.3x cheaper than native fp32 matmul (4 cycles/row).
"""

import numpy as np

F16 = np.float16
SPLIT = 2048.0        # lo-term scale (2^11)
SINV = 1.0 / SPLIT

# Problem shapes (hardcoded per contract -- kernel.py must be self-contained).
B = 1024          # batch = s_num * w_num
S = 126           # state size
R = 2             # reward size
D = 128           # S + R
H1 = 5120         # 40*D
H2 = 7680         # 60*D
A = 16            # actions
AR = A * R        # 32
W_NUM = 4
S_NUM = B // W_NUM  # 256

NCORES = 8
SLICE = H2 // NCORES       # 960 true cols per core
NCT = 8                    # col tiles per core (padded 960 -> 1024)
CPAD = NCT * 128           # 1024
NKT = H1 // 128            # 40 k-tiles
NBT = B // 128             # 8 batch tiles
KCH = 10                   # kt chunk size for weight streaming
NCH = NKT // KCH           # 4 chunks

_PROGRAM = None  # compiled Bass program cache (per process)


def _build_program(sim_variant=False):
    # sim_variant=True: single-core, collective replaced by a DMA bounce, for
    # TimelineSim cost-model analysis (TimelineSim supports neither).
    import concourse.bass as bass  # noqa: F401
    import concourse.bacc as bacc
    import concourse.mybir as mybir
    from concourse import tile
    from contextlib import ExitStack

    f32 = mybir.dt.float32
    f16 = mybir.dt.float16
    AF = mybir.ActivationFunctionType
    OP = mybir.AluOpType
    AX = mybir.AxisListType

    nc = bacc.Bacc(
        "TRN2",
        target_bir_lowering=False,
        debug=False,
        enable_asserts=False,
        num_devices=1 if sim_variant else NCORES,
    )

    # Per-core inputs (same names on every core; data differs per core).
    xh = nc.dram_tensor("xh", [D, B], f16, kind="ExternalInput")
    xl = nc.dram_tensor("xl", [D, B], f16, kind="ExternalInput")
    w1h = nc.dram_tensor("w1h", [D, H1], f16, kind="ExternalInput")
    w1l = nc.dram_tensor("w1l", [D, H1], f16, kind="ExternalInput")
    b1r = nc.dram_tensor("b1r", [128, NKT], f32, kind="ExternalInput")
    wgh = nc.dram_tensor("wgh", [NCT, 128, NKT, 128], f16, kind="ExternalInput")
    wgl = nc.dram_tensor("wgl", [NCT, 128, NKT, 128], f16, kind="ExternalInput")
    w2h = nc.dram_tensor("w2h", [NCT, 128, NKT, 128], f16, kind="ExternalInput")
    w2l = nc.dram_tensor("w2l", [NCT, 128, NKT, 128], f16, kind="ExternalInput")
    bgs = nc.dram_tensor("bgs", [128, NCT], f32, kind="ExternalInput")
    b2s = nc.dram_tensor("b2s", [128, NCT], f32, kind="ExternalInput")
    w3h = nc.dram_tensor("w3h", [128, NCT, AR], f16, kind="ExternalInput")
    w3l = nc.dram_tensor("w3l", [128, NCT, AR], f16, kind="ExternalInput")
    b3b = nc.dram_tensor("b3b", [128, AR], f32, kind="ExternalInput")
    prefc = nc.dram_tensor("prefc", [128, R], f32, kind="ExternalInput")
    sel = nc.dram_tensor("sel", [128, 1], f32, kind="ExternalInput")

    q_out = nc.dram_tensor("q_out", [128, NBT, AR], f32, kind="ExternalOutput")
    hq_out = nc.dram_tensor("hq_out", [128, R], f32, kind="ExternalOutput")

    with ExitStack() as ctx:
        tc = ctx.enter_context(tile.TileContext(nc))
        const = ctx.enter_context(tc.tile_pool(name="const", bufs=1))
        ps = ctx.enter_context(tc.tile_pool(name="ps", bufs=8, space="PSUM"))
        wpool = ctx.enter_context(tc.tile_pool(name="wpool", bufs=5))
        h1p = ctx.enter_context(tc.tile_pool(name="h1p", bufs=1))
        gh = ctx.enter_context(tc.tile_pool(name="gh", bufs=8))
        hgp = ctx.enter_context(tc.tile_pool(name="hgp", bufs=1))
        hp = ctx.enter_context(tc.tile_pool(name="hp", bufs=1))
        dram = ctx.enter_context(tc.tile_pool(name="dram", bufs=1, space="DRAM"))

        dma = nc.sync.dma_start

        xh_sb = const.tile([D, B], f16, tag="xh", name="xh_sb")
        dma(xh_sb[:], xh[:])
        xl_sb = const.tile([D, B], f16, tag="xl", name="xl_sb")
        dma(xl_sb[:], xl[:])
        w1h_sb = const.tile([D, H1], f16, tag="w1h", name="w1h_sb")
        w1l_sb = const.tile([D, H1], f16, tag="w1l", name="w1l_sb")

        def w1_chunk(wch):
            wcs = slice(wch * KCH * 128, (wch + 1) * KCH * 128)
            dma(w1h_sb[:, wcs], w1h[:, wcs])
            dma(w1l_sb[:, wcs], w1l[:, wcs])

        w1_chunk(0)
        b1_sb = const.tile([128, NKT], f32, tag="b1", name="b1_sb")
        dma(b1_sb[:], b1r[:])
        bgs_sb = const.tile([128, NCT], f32, tag="bgs", name="bgs_sb")
        dma(bgs_sb[:], bgs[:])
        b2s_sb = const.tile([128, NCT], f32, tag="b2s", name="b2s_sb")
        dma(b2s_sb[:], b2s[:])
        w3h_sb = const.tile([128, NCT, AR], f16, tag="w3h", name="w3h_sb")
        dma(w3h_sb[:], w3h[:])
        w3l_sb = const.tile([128, NCT, AR], f16, tag="w3l", name="w3l_sb")
        dma(w3l_sb[:], w3l[:])
        b3_sb = const.tile([128, AR], f32, tag="b3", name="b3_sb")
        dma(b3_sb[:], b3b[:])
        pref_sb = const.tile([128, R], f32, tag="pref", name="pref_sb")
        dma(pref_sb[:], prefc[:])
        sel_sb = const.tile([128, 1], f32, tag="sel", name="sel_sb")
        dma(sel_sb[:], sel[:])

        qpart = const.tile([128, NBT, AR], f32, tag="qpart", name="qpart")
        qf = const.tile([128, NBT, AR], f32, tag="qf", name="qf")

        hg_by_half = []
        q_in = [None, None]
        q_rd = [None, None]

        def phase3_bt(h, bl):
            # partial q[batch, 32] for batch tile h*4+bl of half h
            hgh_t, hgl_t = hg_by_half[h]
            bt = h * 4 + bl
            bs = slice(bl * 128, (bl + 1) * 128)
            pqh = ps.tile([128, AR], f32, tag="ps", name="pqh")
            pql = ps.tile([128, AR], f32, tag="ps", name="pql")
            for ct in range(NCT):
                st = ct == 0
                sp = ct == NCT - 1
                nc.tensor.matmul(
                    pqh[:], hgh_t[:, ct, bs], w3h_sb[:, ct, :], start=st, stop=sp
                )
                nc.tensor.matmul(
                    pql[:], hgl_t[:, ct, bs], w3h_sb[:, ct, :], start=st, stop=False
                )
                nc.tensor.matmul(
                    pql[:], hgh_t[:, ct, bs], w3l_sb[:, ct, :], start=False, stop=sp
                )
            qlo = gh.tile([128, AR], f32, tag="tmp", name="qlo")
            nc.scalar.activation(qlo[:], pql[:], AF.Copy, bias=0.0, scale=SINV)
            nc.vector.tensor_tensor(qpart[:, bt, :], pqh[:], qlo[:], OP.add)

        def all_reduce(h):
            # AllReduce this half's partial q [128, 4, 32] across the 8 cores
            q_in[h] = dram.tile([128, 4, AR], f32, tag=f"qin{h}", name=f"q_in{h}")
            q_rd[h] = dram.tile(
                [128, 4, AR], f32, tag=f"qrd{h}", name=f"q_rd{h}",
                addr_space="Shared",
            )
            dma(q_in[h][:], qpart[:, h * 4:(h + 1) * 4, :])
            if sim_variant:
                dma(q_rd[h][:], q_in[h][:])
            else:
                nc.gpsimd.collective_compute(
                    "AllReduce",
                    OP.add,
                    replica_groups=[list(range(NCORES))],
                    ins=[q_in[h].opt()],
                    outs=[q_rd[h].opt()],
                )

        for half in range(2):
            hb = slice(half * 512, (half + 1) * 512)

            # ---- Phases 1+2 interleaved. Phase 1 (h1 = relu(W1.T x + b1),
            # fp16 hi + scaled lo) stalls PE on its PSUM-evacuation chain if
            # emitted alone, so phase-2 ct=0 matmuls for already-evacuated
            # k-tiles are interleaved to keep PE fed.
            h1h = h1p.tile([128, NKT, 512], f16, tag="h1h", name="h1h")
            h1l = h1p.tile([128, NKT, 512], f16, tag="h1l", name="h1l")
            hgh = hgp.tile([128, NCT, 512], f16, tag="hgh", name="hgh")
            hgl = hgp.tile([128, NCT, 512], f16, tag="hgl", name="hgl")
            hg_by_half.append((hgh, hgl))

            psums = {}
            wtiles = {}

            def p2_weights(ct, ch):
                cs = slice(ch * KCH, (ch + 1) * KCH)
                wgh_t = wpool.tile([128, KCH, 128], f16, tag="wgh", name="wgh_t")
                dma(wgh_t[:], wgh[ct, :, cs, :])
                wgl_t = wpool.tile([128, KCH, 128], f16, tag="wgl", name="wgl_t")
                dma(wgl_t[:], wgl[ct, :, cs, :])
                w2h_t = wpool.tile([128, KCH, 128], f16, tag="w2h", name="w2h_t")
                dma(w2h_t[:], w2h[ct, :, cs, :])
                w2l_t = wpool.tile([128, KCH, 128], f16, tag="w2l", name="w2l_t")
                dma(w2l_t[:], w2l[ct, :, cs, :])
                wtiles[ct, ch] = (wgh_t, wgl_t, w2h_t, w2l_t)

            def p2_alloc(ct):
                psums[ct] = (
                    ps.tile([128, 512], f32, tag="ps", name="pgh"),
                    ps.tile([128, 512], f32, tag="ps", name="pgl"),
                    ps.tile([128, 512], f32, tag="ps", name="phh"),
                    ps.tile([128, 512], f32, tag="ps", name="phl"),
                )

            def p2_mms(ct, kt):
                pgh, pgl, phh, phl = psums[ct]
                wgh_t, wgl_t, w2h_t, w2l_t = wtiles[ct, kt // KCH]
                kl = kt % KCH
                st = kt == 0
                sp = kt == NKT - 1
                rh = h1h[:, kt, :]
                rl = h1l[:, kt, :]
                nc.tensor.matmul(pgh[:], wgh_t[:, kl, :], rh, start=st, stop=sp)
                nc.tensor.matmul(pgl[:], wgh_t[:, kl, :], rl, start=st, stop=False)
                nc.tensor.matmul(pgl[:], wgl_t[:, kl, :], rh, start=False, stop=sp)
                nc.tensor.matmul(phh[:], w2h_t[:, kl, :], rh, start=st, stop=sp)
                nc.tensor.matmul(phl[:], w2h_t[:, kl, :], rl, start=st, stop=False)
                nc.tensor.matmul(phl[:], w2l_t[:, kl, :], rh, start=False, stop=sp)

            LAG = 3 if half == 0 else 7
            p2_weights(0, 0)
            for kt in range(NKT):
                ks = slice(kt * 128, (kt + 1) * 128)
                if half == 0 and kt % KCH == KCH - 3 and kt // KCH + 1 < NCH:
                    w1_chunk(kt // KCH + 1)
                p1h = ps.tile([128, 512], f32, tag="ps", name="p1h")
                p1l = ps.tile([128, 512], f32, tag="ps", name="p1l")
                nc.tensor.matmul(
                    p1h[:], w1h_sb[:, ks], xh_sb[:, hb], start=True, stop=True
                )
                nc.tensor.matmul(
                    p1l[:], w1h_sb[:, ks], xl_sb[:, hb], start=True, stop=False
                )
                nc.tensor.matmul(
                    p1l[:], w1l_sb[:, ks], xh_sb[:, hb], start=False, stop=True
                )
                lo = gh.tile([128, 512], f32, tag="tmp", name="lo")
                nc.scalar.activation(lo[:], p1l[:], AF.Copy, bias=0.0, scale=SINV)
                zb = gh.tile([128, 512], f32, tag="tmp", name="zb")
                nc.vector.scalar_tensor_tensor(
                    zb[:], p1h[:], b1_sb[:, kt:kt + 1], lo[:], OP.add, OP.add
                )
                nc.scalar.activation(h1h[:, kt, :], zb[:], AF.Relu)
                dd = gh.tile([128, 512], f32, tag="tmp", name="dd")
                nc.vector.scalar_tensor_tensor(
                    dd[:], zb[:], 0.0, h1h[:, kt, :], OP.max, OP.subtract
                )
                nc.vector.tensor_scalar_mul(h1l[:, kt, :], dd[:], SPLIT)
                if half == 1 and kt < 4:
                    # previous half's W3-stage, interleaved as PE gap filler
                    phase3_bt(0, kt)
                if half == 1 and kt == 4:
                    all_reduce(0)
                k2 = kt - LAG
                if k2 >= 0:
                    if k2 == 0:
                        p2_alloc(0)
                    if k2 % KCH == 0 and k2 // KCH + 1 < NCH:
                        p2_weights(0, k2 // KCH + 1)
                    p2_mms(0, k2)
            for k2 in range(NKT - LAG, NKT):
                p2_mms(0, k2)

            # ---- Phase 2 (remaining col tiles) ----
            for ct in range(NCT):
                if ct > 0:
                    p2_alloc(ct)
                    for ch in range(NCH):
                        p2_weights(ct, ch)
                        for kl in range(KCH):
                            p2_mms(ct, ch * KCH + kl)
                pgh, pgl, phh, phl = psums[ct]
                glo = gh.tile([128, 512], f32, tag="tmp", name="glo")
                nc.scalar.activation(glo[:], pgl[:], AF.Copy, bias=0.0, scale=SINV)
                zg = gh.tile([128, 512], f32, tag="tmp", name="zg")
                nc.vector.tensor_tensor(zg[:], pgh[:], glo[:], OP.add)
                g_sb = gh.tile([128, 512], f32, tag="tmp", name="g_sb")
                nc.scalar.activation(
                    g_sb[:], zg[:], AF.Sigmoid, bias=bgs_sb[:, ct:ct + 1], scale=500.0
                )
                hlo = gh.tile([128, 512], f32, tag="tmp", name="hlo")
                nc.scalar.activation(hlo[:], phl[:], AF.Copy, bias=0.0, scale=SINV)
                zh2 = gh.tile([128, 512], f32, tag="tmp", name="zh2")
                nc.vector.scalar_tensor_tensor(
                    zh2[:], phh[:], b2s_sb[:, ct:ct + 1], hlo[:], OP.add, OP.add
                )
                hgf = gh.tile([128, 512], f32, tag="tmp", name="hgf")
                nc.vector.scalar_tensor_tensor(
                    hgf[:], zh2[:], 0.0, g_sb[:], OP.max, OP.mult
                )
                nc.scalar.activation(hgh[:, ct, :], hgf[:], AF.Copy)
                dd2 = gh.tile([128, 512], f32, tag="tmp", name="dd2")
                nc.vector.tensor_tensor(dd2[:], hgf[:], hgh[:, ct, :], OP.subtract)
                nc.vector.tensor_scalar_mul(hgl[:, ct, :], dd2[:], SPLIT)

        # ---- last half's W3-stage + its AllReduce (tail) ----
        for bl in range(4):
            phase3_bt(1, bl)

        # ---- Phase 4b: second half's AllReduce (first was overlapped) ----
        all_reduce(1)
        dma(qf[:, 0:4, :], q_rd[0][:])
        dma(qf[:, 4:8, :], q_rd[1][:])
        for bt in range(NBT):
            nc.vector.tensor_tensor(qf[:, bt, :], qf[:, bt, :], b3_sb[:], OP.add)
        dma(q_out[:], qf[:])

        # ---- Phase 5: H() top-1 selection for this core's 128 rows ----
        # Row p = c*128+pp belongs to state group (c%2)*128+pp; its candidates
        # are q tiles {par, par+2, par+4, par+6} with par = c%2. The program is
        # SPMD so we compute both parities and blend with the per-core `sel`.
        # candidate views: qf [128, 8, 32] -> [128, 4(j), 2(par), 16(a), 2(r)]
        qv = qf.rearrange("p (u v) (a r) -> p u v a r", v=2, r=R)
        hq_par = []
        for par in range(2):
            v0 = qv[:, :, par, :, 0]                      # [128, 4, 16]
            v1 = qv[:, :, par, :, 1]
            t1 = hp.tile([128, 64], f32, tag=f"t1{par}", name=f"t1_{par}")
            t1v = t1.rearrange("p (u a) -> p u a", u=W_NUM)
            nc.vector.tensor_scalar_mul(t1v, v1, pref_sb[:, 1:2])
            sc = hp.tile([128, 64], f32, tag=f"sc{par}", name=f"sc_{par}")
            scv = sc.rearrange("p (u a) -> p u a", u=W_NUM)
            nc.vector.scalar_tensor_tensor(
                scv, v0, pref_sb[:, 0:1], t1v, OP.mult, OP.add
            )
            m = hp.tile([128, 1], f32, tag=f"m{par}", name=f"m_{par}")
            nc.vector.reduce_max(m[:], sc[:], axis=AX.X)
            mask = hp.tile([128, 64], f32, tag=f"mask{par}", name=f"mask_{par}")
            nc.vector.tensor_scalar(mask[:], sc[:], m[:], None, OP.is_equal)
            maskv = mask.rearrange("p (u a) -> p u a", u=W_NUM)
            den = hp.tile([128, 1], f32, tag=f"den{par}", name=f"den_{par}")
            nc.vector.reduce_sum(den[:], mask[:], axis=AX.X)
            rden = hp.tile([128, 1], f32, tag=f"rden{par}", name=f"rden_{par}")
            nc.vector.reciprocal(rden[:], den[:])
            hqp = hp.tile([128, R], f32, tag=f"hqp{par}", name=f"hqp_{par}")
            for r, vv in enumerate((v0, v1)):
                mc = hp.tile([128, 64], f32, tag=f"mc{par}", name=f"mc_{par}_{r}")
                mcv = mc.rearrange("p (u a) -> p u a", u=W_NUM)
                nc.vector.tensor_tensor(mcv, maskv, vv, OP.mult)
                nsum = hp.tile([128, 1], f32, tag=f"ns{par}{r}", name=f"ns_{par}_{r}")
                nc.vector.reduce_sum(nsum[:], mc[:], axis=AX.X)
                nc.vector.tensor_tensor(hqp[:, r:r + 1], nsum[:], rden[:], OP.mult)
            hq_par.append(hqp)
        dif = hp.tile([128, R], f32, tag="dif", name="dif")
        nc.vector.tensor_tensor(dif[:], hq_par[0][:], hq_par[1][:], OP.subtract)
        hq_sb = hp.tile([128, R], f32, tag="hq_sb", name="hq_sb")
        nc.vector.scalar_tensor_tensor(
            hq_sb[:], dif[:], sel_sb[:, 0:1], hq_par[1][:], OP.mult, OP.add
        )
        dma(hq_out[:], hq_sb[:])

    nc.compile()
    return nc


def _split(a):
    """fp16 hi + 2^11-scaled fp16 lo decomposition of a float32 array."""
    hi = a.astype(F16)
    lo = ((a - hi.astype(np.float32)) * SPLIT).astype(F16)
    return hi, lo


def _prep_inputs(state, preference, W1, b1, Wg, bg, W2, b2, W3, b3):
    """Build the 8 per-core input maps (host-side shard + layout prep)."""
    state = np.asarray(state, np.float32)
    preference = np.asarray(preference, np.float32)
    W1 = np.asarray(W1, np.float32)
    b1 = np.asarray(b1, np.float32)
    Wg = np.asarray(Wg, np.float32)
    bg = np.asarray(bg, np.float32)
    W2 = np.asarray(W2, np.float32)
    b2 = np.asarray(b2, np.float32)
    W3 = np.asarray(W3, np.float32)
    b3 = np.asarray(b3, np.float32)

    x = np.concatenate([state, preference], axis=1)          # [B, D]
    xh, xl = _split(np.ascontiguousarray(x.T))               # [D, B]
    w1h, w1l = _split(W1)                                    # [D, H1]
    b1r = np.ascontiguousarray(b1.reshape(NKT, 128).T).astype(np.float32)
    b3b = np.ascontiguousarray(np.broadcast_to(b3, (128, AR))).astype(np.float32)

    def tile_kc(wp):
        # [H1, CPAD] -> [ct, k, kt, c] so each (ct, kt-chunk) DMA is contiguous
        return np.ascontiguousarray(
            wp.reshape(NKT, 128, NCT, 128).transpose(2, 1, 0, 3)
        )

    in_maps = []
    for c in range(NCORES):
        sl = slice(c * SLICE, (c + 1) * SLICE)

        wgp = np.zeros((H1, CPAD), np.float32)
        wgp[:, :SLICE] = Wg[:, sl]
        gh_, gl_ = _split(wgp)
        wgh = tile_kc(gh_)
        wgl = tile_kc(gl_)

        w2p = np.zeros((H1, CPAD), np.float32)
        w2p[:, :SLICE] = W2[:, sl]
        hh_, hl_ = _split(w2p)
        w2h = tile_kc(hh_)
        w2l = tile_kc(hl_)

        bgp = np.zeros((CPAD,), np.float32)
        bgp[:SLICE] = 500.0 * bg[sl]
        bgs = np.ascontiguousarray(bgp.reshape(NCT, 128).T).astype(np.float32)

        b2p = np.zeros((CPAD,), np.float32)
        b2p[:SLICE] = b2[sl]
        b2s = np.ascontiguousarray(b2p.reshape(NCT, 128).T).astype(np.float32)

        w3p = np.zeros((CPAD, AR), np.float32)
        w3p[:SLICE] = W3[sl, :]
        w3hi, w3lo = _split(w3p)
        w3h = np.ascontiguousarray(w3hi.reshape(NCT, 128, AR).transpose(1, 0, 2))
        w3l = np.ascontiguousarray(w3lo.reshape(NCT, 128, AR).transpose(1, 0, 2))

        in_maps.append({
            "xh": xh, "xl": xl,
            "w1h": w1h, "w1l": w1l,
            "b1r": b1r,
            "wgh": wgh, "wgl": wgl,
            "w2h": w2h, "w2l": w2l,
            "bgs": bgs, "b2s": b2s,
            "w3h": w3h, "w3l": w3l,
            "b3b": b3b,
            "prefc": np.ascontiguousarray(preference[c * 128:(c + 1) * 128]),
            "sel": np.full((128, 1), 1.0 if c % 2 == 0 else 0.0, np.float32),
        })
    return in_maps


def _emulate_core(m):
    """Numpy emulation of the device program for one core (layout check)."""
    f32 = np.float32

    def mm3(ah, al, bh, bl):
        # ah/bh fp16 hi, al/bl fp16 lo*2^11; fp32 accumulation
        hi = ah.astype(f32) @ bh.astype(f32)
        lo = al.astype(f32) @ bh.astype(f32) + ah.astype(f32) @ bl.astype(f32)
        return hi + SINV * lo

    h1T = np.zeros((NKT, 128, B), f32)
    for kt in range(NKT):
        ks = slice(kt * 128, (kt + 1) * 128)
        z = mm3(
            m["w1h"][:, ks].T, m["w1l"][:, ks].T,
            m["xh"], m["xl"],
        )
        h1T[kt] = np.maximum(z + m["b1r"][:, kt:kt + 1], 0.0)
    h1h = h1T.astype(F16)
    h1l = ((h1T - h1h.astype(f32)) * SPLIT).astype(F16)

    hgh = np.zeros((NCT, 128, B), F16)
    hgl = np.zeros((NCT, 128, B), F16)
    for ct in range(NCT):
        zg = np.zeros((128, B), f32)
        zh = np.zeros((128, B), f32)
        for kt in range(NKT):
            zg += mm3(m["wgh"][ct, :, kt, :].T, m["wgl"][ct, :, kt, :].T,
                      h1h[kt], h1l[kt])
            zh += mm3(m["w2h"][ct, :, kt, :].T, m["w2l"][ct, :, kt, :].T,
                      h1h[kt], h1l[kt])
        g = 1.0 / (1.0 + np.exp(np.clip(-(zg * 500.0 + m["bgs"][:, ct:ct + 1]),
                                        -80, 80)))
        h2 = np.maximum(zh + m["b2s"][:, ct:ct + 1], 0.0)
        hg = h2 * g
        hgh[ct] = hg.astype(F16)
        hgl[ct] = ((hg - hgh[ct].astype(f32)) * SPLIT).astype(F16)

    qpart = np.zeros((128, NBT, AR), f32)
    for bt in range(NBT):
        bs = slice(bt * 128, (bt + 1) * 128)
        acc = np.zeros((128, AR), f32)
        for ct in range(NCT):
            acc += mm3(hgh[ct][:, bs].T, hgl[ct][:, bs].T,
                       m["w3h"][:, ct, :], m["w3l"][:, ct, :])
        qpart[:, bt, :] = acc
    return qpart


def _emulate_h(qf, m):
    """Numpy emulation of phase 5 for one core. qf: [128, NBT, AR] w/ b3."""
    f32 = np.float32
    hq_par = []
    for par in range(2):
        c0 = np.zeros((128, 64), f32)
        c1 = np.zeros((128, 64), f32)
        for j in range(W_NUM):
            src = qf[:, par + 2 * j, :].reshape(128, A, R)
            c0[:, j * A:(j + 1) * A] = src[:, :, 0]
            c1[:, j * A:(j + 1) * A] = src[:, :, 1]
        sc = c0 * m["prefc"][:, 0:1] + c1 * m["prefc"][:, 1:2]
        mx = sc.max(axis=1, keepdims=True)
        mask = (sc == mx).astype(f32)
        den = mask.sum(axis=1, keepdims=True)
        hq0 = (mask * c0).sum(axis=1, keepdims=True) / den
        hq1 = (mask * c1).sum(axis=1, keepdims=True) / den
        hq_par.append(np.concatenate([hq0, hq1], axis=1))
    sel = m["sel"][:, 0:1]
    return hq_par[1] + sel * (hq_par[0] - hq_par[1])


def _run_emulated(in_maps):
    qparts = [_emulate_core(m) for m in in_maps]
    qf = np.sum(qparts, axis=0, dtype=np.float32)
    qf = qf + in_maps[0]["b3b"][0][None, None, :]
    hq = np.concatenate([_emulate_h(qf, m) for m in in_maps], axis=0)
    q = qf.transpose(1, 0, 2).reshape(B, AR)
    return hq.astype(np.float32), q.reshape(B, A, R).astype(np.float32)


def kernel(state, preference, W1, b1, Wg, bg, W2, b2, W3, b3, w_num,
           _emulate=False):
    assert int(w_num) == W_NUM
    in_maps = _prep_inputs(state, preference, W1, b1, Wg, bg, W2, b2, W3, b3)

    if _emulate:
        return _run_emulated(in_maps)

    global _PROGRAM
    if _PROGRAM is None:
        _PROGRAM = _build_program()

    from concourse.bass_utils import run_bass_kernel_spmd

    res = run_bass_kernel_spmd(_PROGRAM, in_maps, core_ids=list(range(NCORES)))
    hq = np.concatenate(
        [res.results[c]["hq_out"] for c in range(NCORES)], axis=0
    ).astype(np.float32)
    q = (
        res.results[0]["q_out"]
        .transpose(1, 0, 2)
        .reshape(B, A, R)
        .astype(np.float32)
    )
    return hq, q
